# revision 1
# baseline (speedup 1.0000x reference)
"""XL-BOMD rank-4 Krylov propagation (EnergyXL) on 8 TRN2 NeuronCores.

Data-parallel over molecules: 512 mols -> 64 per core. Per molecule
(N=192, rank=4):
  dDS = D - P ; v0 = dDS/||dDS||
  for k in 0..3:  T = v_k R ; W_k = R T - v_k   (PE sandwiches, fp32)
                  v_{k+1} = GS-orthonormalize(W_k vs v_0..v_k)
  O[IJ] = <W_I,W_J>, c[J] = <W_J,dDS>  (Frobenius ips, DVE TTR)
  y = O^-1 c (batched symmetric Gauss elim over mol-partition layout)
  out = -sum_I y_I V_I

Matrices live in SBUF as hi [128,192] (rows 0:128) + lo [64,192]
(rows 128:192) fp32 tiles.  Inner products use fused
tensor_tensor_reduce with hi->lo accumulator chaining; cross-partition
sums + per-mol scalar broadcast via ones-matmul on the PE.
"""

import os
import sys

sys.path.insert(0, "/opt/trn_rl_repo")

import numpy as np

import concourse.bass as bass
import concourse.bacc as bacc
import concourse.tile as tile
from concourse import mybir
from concourse.bass_utils import run_bass_kernel_spmd

F32 = mybir.dt.float32
ALU = mybir.AluOpType
ACTF = mybir.ActivationFunctionType

NMOL, N, RANK = 512, 192, 4
NCORES = 8
MPC = NMOL // NCORES  # 64 molecules per core
HI, LO = 128, 64
BLK = 8  # molecules per solve block

# partials column map (per molecule, [128, 48] tile); every quantity is a
# (hi, lo) column pair summed post-broadcast (strided pair-add).
# Layout is rank-contiguous so ONE ones-matmul per rank broadcasts
# O_kk + c_k + GS coefs together:
#   col 0,1: ||dDS||^2
#   rank-k block at RBASE[k]: O(k,0..k) pairs, c_k pair, GS(k+1, 0..k) pairs
COL_N0 = 0
RBASE = [2, 8, 18, 32]          # rank block bases; sizes 6,10,14,10
NPART = 48
# post-gather pair-added column indices (gather covers cols 2..41 -> 20 pairs)
GIX = {"O00": 0, "c0": 1, "G10": 2, "O10": 3, "O11": 4, "c1": 5, "G20": 6,
       "G21": 7, "O20": 8, "O21": 9, "O22": 10, "c2": 11, "G30": 12,
       "G31": 13, "G32": 14, "O30": 15, "O31": 16, "O32": 17, "O33": 18,
       "c3": 19}


def _o_col(k, j):
    assert j <= k
    return RBASE[k] + 2 * j


def _c_col(k):
    return RBASE[k] + 2 * (k + 1)


def _gs_col(kk, j):
    # coef <w_k, v_j> for v_kk (kk = k+1), stored in rank-k block
    k = kk - 1
    return RBASE[k] + 2 * (k + 1) + 2 + 2 * j


def build_core_kernel(n_mols=MPC):
    nc = bacc.Bacc(None, target_bir_lowering=False, enable_partition_id=False)
    D = nc.dram_tensor("D", [n_mols, N, N], F32, kind="ExternalInput")
    P = nc.dram_tensor("P", [n_mols, N, N], F32, kind="ExternalInput")
    R = nc.dram_tensor("Rm", [n_mols, N, N], F32, kind="ExternalInput")
    OUT = nc.dram_tensor("OUT", [n_mols, N, N], F32, kind="ExternalOutput")

    with tile.TileContext(nc) as tc:
        _body(nc, tc, D, P, R, OUT)
    nc.finalize()
    return nc


def _body(nc, tc, D, P, R, OUT):
    import contextlib

    ctx = contextlib.ExitStack()
    with ctx:
        consts = ctx.enter_context(tc.tile_pool(name="consts", bufs=1))
        persist = ctx.enter_context(tc.tile_pool(name="persist", bufs=11))
        fast = ctx.enter_context(tc.tile_pool(name="fast", bufs=6))
        temps = ctx.enter_context(tc.tile_pool(name="temps", bufs=8))
        scal = ctx.enter_context(tc.tile_pool(name="scal", bufs=16))
        blkp = ctx.enter_context(tc.tile_pool(name="blkp", bufs=2))
        ps_mm = ctx.enter_context(tc.tile_pool(name="ps_mm", bufs=2, space="PSUM"))
        ps_sm = ctx.enter_context(tc.tile_pool(name="ps_sm", bufs=2, space="PSUM"))
        ps_g = ctx.enter_context(tc.tile_pool(name="ps_g", bufs=2, space="PSUM"))

        # --- constants ---
        ones = consts.tile([HI, HI], F32)      # all-ones for partition-sum / bcast matmuls
        nc.vector.memset(ones, 1.0)
        sel = consts.tile([HI, 2 * LO - 1], F32)  # windowed one-hot column selector
        nc.vector.memset(sel, 0.0)
        nc.vector.memset(sel[:, LO - 1 : LO], 1.0)
        id8 = consts.tile([BLK, BLK], F32)     # identity for y row-masking
        idt = consts.tile([BLK, BLK], mybir.dt.int32)
        nc.gpsimd.iota(idt, pattern=[[-1, BLK]], base=0, channel_multiplier=1)
        nc.vector.tensor_scalar(out=id8, in0=idt, scalar1=0, scalar2=None,
                                op0=ALU.is_equal)

        n_mols = D.shape[0]
        for b in range(n_mols // BLK):
            mols = list(range(b * BLK, (b + 1) * BLK))
            blk_state = []
            for m in mols:
                st = _mol_pipeline(nc, tc, D, P, R, m, persist, fast, temps, scal,
                                   ps_mm, ps_sm, ones)
                blk_state.append(st)
            _block_tail(nc, tc, OUT, b, mols, blk_state, consts, fast, temps, scal,
                        blkp, ps_g, ps_sm, ones, sel, id8)


def _ip(nc, partials, col2, a, b_, scr, scr2, mult_eng="dve", red="act"):
    """<A,B> Frobenius: elementwise mult then free-dim reduce into col pair."""
    a_hi, a_lo = a
    b_hi, b_lo = b_
    scr_hi, scr_lo = scr
    me = nc.vector if mult_eng == "dve" else nc.gpsimd
    me.tensor_mul(scr_hi, a_hi, b_hi)
    me.tensor_mul(scr_lo, a_lo, b_lo)
    if red == "act":
        nc.scalar.activation(out=scr_hi, in_=scr_hi, func=ACTF.Copy,
                             accum_out=partials[:, col2 : col2 + 1])
        nc.scalar.activation(out=scr_lo, in_=scr_lo, func=ACTF.Copy,
                             accum_out=partials[:LO, col2 + 1 : col2 + 2])
    else:
        nc.vector.tensor_reduce(out=partials[:, col2 : col2 + 1], in_=scr_hi,
                                axis=mybir.AxisListType.X, op=ALU.add)
        nc.vector.tensor_reduce(out=partials[:LO, col2 + 1 : col2 + 2], in_=scr_lo,
                                axis=mybir.AxisListType.X, op=ALU.add)


def _norm_sq(nc, partials, col2, x, scr_hi, scr_lo):
    """||X||^2 on ACT: square+accumulate, hi/lo to cols col2, col2+1."""
    x_hi, x_lo = x
    nc.scalar.activation(out=scr_hi, in_=x_hi, func=ACTF.Square,
                         accum_out=partials[:, col2 : col2 + 1])
    nc.scalar.activation(out=scr_lo, in_=x_lo, func=ACTF.Square,
                         accum_out=partials[:LO, col2 + 1 : col2 + 2])


def _bcast(nc, ps_sm, ones, partials, col, ncols):
    """ones-matmul: col sums of partials[:, col:col+ncols] broadcast to 128 partitions."""
    bc = ps_sm.tile([HI, ncols], F32, tag="bc")
    nc.tensor.matmul(bc, lhsT=ones, rhs=partials[:, col : col + ncols],
                     start=True, stop=True)
    return bc


def _inv_norm(nc, scal, ps_sm, ones, partials, col2, tag):
    """1/sqrt(hi_col + lo_col) as a [128,1] SBUF tile."""
    bc = _bcast(nc, ps_sm, ones, partials, col2, 2)
    s = scal.tile([HI, 5], F32, tag=tag)
    nc.scalar.copy(s[:, 0:2], bc)
    nc.vector.tensor_add(s[:, 2:3], s[:, 0:1], s[:, 1:2])
    nc.scalar.sqrt(s[:, 3:4], s[:, 2:3])
    nc.vector.reciprocal(s[:, 4:5], s[:, 3:4])
    return s[:, 4:5]


def _sandwich(nc, ps_mm, out_sb, lhsT, rhs):
    """out = lhsT^T @ rhs for 192x192 operands in hi/lo tiles -> PSUM pair."""
    l_hi, l_lo = lhsT
    r_hi, r_lo = rhs
    o_hi = ps_mm.tile([HI, N], F32, tag="mm_hi")
    o_lo = ps_mm.tile([LO, N], F32, tag="mm_lo")
    nc.tensor.matmul(o_hi, lhsT=l_hi[:, 0:HI], rhs=r_hi, start=True, stop=False)
    nc.tensor.matmul(o_hi, lhsT=l_lo[:, 0:HI], rhs=r_lo, start=False, stop=True)
    nc.tensor.matmul(o_lo, lhsT=l_hi[:, HI:N], rhs=r_hi, start=True, stop=False)
    nc.tensor.matmul(o_lo, lhsT=l_lo[:, HI:N], rhs=r_lo, start=False, stop=True)
    return o_hi, o_lo


def _mol_pipeline(nc, tc, D, P, R, m, persist, fast, temps, scal, ps_mm, ps_sm, ones):
    """Emit one molecule's Krylov chain. Returns state dict for the block tail."""
    # --- load R, D, P ---
    r_hi = fast.tile([HI, N], F32, tag="r_hi")
    r_lo = fast.tile([LO, N], F32, tag="r_lo")
    nc.sync.dma_start(out=r_hi, in_=R[m, 0:HI, :])
    nc.sync.dma_start(out=r_lo, in_=R[m, HI:N, :])

    dds_hi = persist.tile([HI, N], F32, tag="dds_hi")
    dds_lo = persist.tile([LO, N], F32, tag="dds_lo")
    nc.sync.dma_start(out=dds_hi, in_=D[m, 0:HI, :])
    nc.sync.dma_start(out=dds_lo, in_=D[m, HI:N, :])
    p_hi = temps.tile([HI, N], F32, tag="p_hi")
    p_lo = temps.tile([LO, N], F32, tag="p_lo")
    nc.sync.dma_start(out=p_hi, in_=P[m, 0:HI, :])
    nc.sync.dma_start(out=p_lo, in_=P[m, HI:N, :])
    nc.gpsimd.tensor_sub(dds_hi, dds_hi, p_hi)
    nc.gpsimd.tensor_sub(dds_lo, dds_lo, p_lo)
    dds = (dds_hi, dds_lo)

    partials = scal.tile([HI, NPART], F32, tag="partials")
    nc.vector.memset(partials, 0.0)

    scr_hi = temps.tile([HI, N], F32, tag="scr_hi")
    scr_lo = temps.tile([LO, N], F32, tag="scr_lo")
    scr = (scr_hi, scr_lo)
    scr2 = None

    # --- v0 = dDS / ||dDS|| ---
    _norm_sq(nc, partials, COL_N0, dds, scr_hi, scr_lo)
    inv0 = _inv_norm(nc, scal, ps_sm, ones, partials, COL_N0, tag="nrm0")
    v_tiles = []
    v0_hi = persist.tile([HI, N], F32, tag="v0_hi")
    v0_lo = persist.tile([LO, N], F32, tag="v0_lo")
    nc.scalar.mul(v0_hi, dds_hi, inv0)
    nc.scalar.mul(v0_lo, dds_lo, inv0[:LO, :])
    v_tiles.append((v0_hi, v0_lo))

    w_tiles = []
    for k in range(RANK):
        vk = v_tiles[k]
        # T = v_k R
        t_ps = _sandwich(nc, ps_mm, None, vk, (r_hi, r_lo))
        t_hi = temps.tile([HI, N], F32, tag="t_hi")
        t_lo = temps.tile([LO, N], F32, tag="t_lo")
        nc.scalar.copy(t_hi, t_ps[0])
        nc.scalar.copy(t_lo, t_ps[1])
        # W_k = R T - v_k
        w_ps = _sandwich(nc, ps_mm, None, (r_hi, r_lo), (t_hi, t_lo))
        w_hi = fast.tile([HI, N], F32, tag=f"w{k}_hi")
        w_lo = fast.tile([LO, N], F32, tag=f"w{k}_lo")
        nc.vector.tensor_sub(w_hi, w_ps[0], vk[0])
        nc.vector.tensor_sub(w_lo, w_ps[1], vk[1])
        wk = (w_hi, w_lo)
        w_tiles.append(wk)

        # O row k and c_k  (off the critical chain)
        for j in range(k + 1):
            _ip(nc, partials, _o_col(k, j), w_tiles[j], wk, scr, scr2,
                mult_eng="dve", red="act")
        _ip(nc, partials, _c_col(k), wk, dds, scr, scr2,
            mult_eng="gpsimd", red="dve")

        # Gram-Schmidt -> v_{k+1}; ||u||^2 = O_kk - sum c_j^2 (no 2nd pass)
        if k < RANK - 1:
            kk = k + 1
            for j in range(kk):
                _ip(nc, partials, _gs_col(kk, j), wk, v_tiles[j], scr, scr2,
                    mult_eng="dve", red="act")
            # one broadcast for O_kk + c_k + GS coefs: cols RBASE[k]+2k ..
            b0 = RBASE[k] + 2 * k
            nb = 4 + 2 * kk
            bc = _bcast(nc, ps_sm, ones, partials, b0, nb)
            s = scal.tile([HI, nb + 2 * kk + 6], F32, tag="gs_s")
            nc.scalar.copy(s[:, 0:nb], bc)
            x = nb
            coefs = s[:, x : x + kk]
            # pair-sum GS cols (offset 4 within block: after O_kk, c_k pairs)
            nc.vector.tensor_add(coefs, s[:, 4 : 4 + 2 * kk : 2],
                                 s[:, 5 : 4 + 2 * kk : 2])
            okk = s[:, x + kk : x + kk + 1]
            # ||u||^2 = (O_kk_hi + O_kk_lo) - sum_j coef_j^2
            u2 = s[:, x + kk + 1 : x + kk + 2]
            sq = s[:, x + kk + 2 : x + kk + 2 + kk]
            nc.vector.tensor_mul(sq, coefs, coefs)
            nc.vector.tensor_reduce(out=u2, in_=sq, axis=mybir.AxisListType.X,
                                    op=ALU.add)
            nc.vector.tensor_add(okk, s[:, 0:1], s[:, 1:2])
            u2b = s[:, x + 2 * kk + 2 : x + 2 * kk + 3]
            nc.vector.tensor_sub(u2b, okk, u2)
            sqr = s[:, x + 2 * kk + 3 : x + 2 * kk + 4]
            nc.scalar.sqrt(sqr, u2b)
            invn = s[:, x + 2 * kk + 4 : x + 2 * kk + 5]
            nc.vector.reciprocal(invn, sqr)

            u_hi = temps.tile([HI, N], F32, tag="u_hi")
            u_lo = temps.tile([LO, N], F32, tag="u_lo")
            for j in range(kk):
                ax_hi = temps.tile([HI, N], F32, tag="ax_hi")
                ax_lo = temps.tile([LO, N], F32, tag="ax_lo")
                nc.scalar.mul(ax_hi, v_tiles[j][0], coefs[:, j : j + 1])
                nc.scalar.mul(ax_lo, v_tiles[j][1], coefs[:LO, j : j + 1])
                src = wk if j == 0 else (u_hi, u_lo)
                nc.gpsimd.tensor_sub(u_hi, src[0], ax_hi)
                nc.gpsimd.tensor_sub(u_lo, src[1], ax_lo)
            vn_hi = persist.tile([HI, N], F32, tag=f"v{kk}_hi")
            vn_lo = persist.tile([LO, N], F32, tag=f"v{kk}_lo")
            nc.scalar.mul(vn_hi, u_hi, invn)
            nc.scalar.mul(vn_lo, u_lo, invn[:LO, :])
            v_tiles.append((vn_hi, vn_lo))

    return {"partials": partials, "v": v_tiles}


def _solve_sym4(nc, g, s):
    """Batched symmetric 4x4 solve on [BLK,1] column APs.

    g: [BLK, 14] tile, cols 0..9 = O (00,10,11,20,21,22,30,31,32,33),
    cols 10..13 = rhs c.  s: [BLK, 16] scratch.  Returns y col APs (in s cols 0..3).
    Mirrors _solve_sym4_np below; keep in sync.
    """
    def col(t, i):
        return t[:, i : i + 1]

    ox = [GIX[q] for q in ("O00", "O10", "O11", "O20", "O21", "O22",
                           "O30", "O31", "O32", "O33")]
    a, bb, e, c, f, h, d, gg, i_, jj = (col(g, i) for i in ox)
    r0, r1, r2, r3 = (col(g, GIX[f"c{i}"]) for i in range(4))
    p0, p1, p2, p3 = (col(s, 4 + i) for i in range(4))
    l1, l2, l3 = (col(s, 8 + i) for i in range(3))
    t0, t1 = col(s, 11), col(s, 12)
    y0, y1, y2, y3 = (col(s, i) for i in range(4))

    mul = nc.vector.tensor_mul
    sub = nc.vector.tensor_sub
    rec = nc.vector.reciprocal

    def upd(x, l, src):  # x -= l*src
        mul(t0, l, src)
        sub(x, x, t0)

    rec(p0, a)
    mul(l1, bb, p0); mul(l2, c, p0); mul(l3, d, p0)
    upd(e, l1, bb); upd(f, l2, bb); upd(gg, l3, bb)
    upd(h, l2, c); upd(i_, l3, c); upd(jj, l3, d)
    upd(r1, l1, r0); upd(r2, l2, r0); upd(r3, l3, r0)

    rec(p1, e)
    mul(l2, f, p1); mul(l3, gg, p1)
    upd(h, l2, f); upd(i_, l3, f); upd(jj, l3, gg)
    upd(r2, l2, r1); upd(r3, l3, r1)

    rec(p2, h)
    mul(l3, i_, p2)
    upd(jj, l3, i_); upd(r3, l3, r2)

    rec(p3, jj)
    mul(y3, r3, p3)
    # back-substitution
    upd(r2, i_, y3); mul(y2, r2, p2)
    upd(r1, f, y2); upd(r1, gg, y3); mul(y1, r1, p1)
    upd(r0, bb, y1); upd(r0, c, y2); upd(r0, d, y3); mul(y0, r0, p0)
    return [y0, y1, y2, y3]


def _solve_sym4_np(G):
    """NumPy mirror of _solve_sym4 for verification. G: [n, 14] -> y [n, 4]."""
    G = G.copy()
    cols = [G[:, i : i + 1] for i in range(14)]
    a, bb, e, c, f, h, d, gg, i_, jj = cols[:10]
    r0, r1, r2, r3 = cols[10:]
    p0 = 1.0 / a
    l1, l2, l3 = bb * p0, c * p0, d * p0
    e = e - l1 * bb; f = f - l2 * bb; gg = gg - l3 * bb
    h = h - l2 * c; i_ = i_ - l3 * c; jj = jj - l3 * d
    r1 = r1 - l1 * r0; r2 = r2 - l2 * r0; r3 = r3 - l3 * r0
    p1 = 1.0 / e
    l2, l3 = f * p1, gg * p1
    h = h - l2 * f; i_ = i_ - l3 * f; jj = jj - l3 * gg
    r2 = r2 - l2 * r1; r3 = r3 - l3 * r1
    p2 = 1.0 / h
    l3 = i_ * p2
    jj = jj - l3 * i_; r3 = r3 - l3 * r2
    p3 = 1.0 / jj
    y3 = r3 * p3
    r2 = r2 - i_ * y3; y2 = r2 * p2
    r1 = r1 - f * y2; r1 = r1 - gg * y3; y1 = r1 * p1
    r0 = r0 - bb * y1; r0 = r0 - c * y2; r0 = r0 - d * y3; y0 = r0 * p0
    return np.concatenate([y0, y1, y2, y3], axis=1)


def _block_tail(nc, tc, OUT, b, mols, blk_state, consts, fast, temps, scal, blkp,
                ps_g, ps_sm, ones, sel, id8):
    # gather each mol's 14 O/c sums into [BLK, 14] via selector matmuls
    gath = ps_g.tile([BLK, 40], F32, tag="gath")
    for j, st in enumerate(blk_state):
        nc.tensor.matmul(gath, lhsT=sel[:, LO - 1 - j : LO - 1 - j + BLK],
                         rhs=st["partials"][:, 2:42],
                         start=(j == 0), stop=(j == len(blk_state) - 1))
    g_pair = blkp.tile([BLK, 40], F32, tag="g_pair")
    nc.scalar.copy(g_pair, gath)
    g_sb = blkp.tile([BLK, 20], F32, tag="g_sb")
    nc.vector.tensor_add(g_sb, g_pair[:, 0:40:2], g_pair[:, 1:40:2])
    s_sb = blkp.tile([BLK, 16], F32, tag="s_sb")
    ys = _solve_sym4(nc, g_sb, s_sb)
    y_sb = blkp.tile([BLK, RANK], F32, tag="y_sb")
    for i in range(RANK):
        nc.vector.tensor_copy(y_sb[:, i : i + 1], ys[i])

    for j, (m, st) in enumerate(zip(mols, blk_state)):
        ymask = scal.tile([BLK, RANK], F32, tag="ymask")
        nc.vector.tensor_scalar(out=ymask, in0=y_sb, scalar1=id8[:, j : j + 1],
                                scalar2=None, op0=ALU.mult)
        ybc = ps_sm.tile([HI, RANK], F32, tag="bc")
        nc.tensor.matmul(ybc, lhsT=ones[0:BLK, :], rhs=ymask, start=True, stop=True)
        yb = scal.tile([HI, RANK], F32, tag="yb")
        nc.scalar.copy(yb, ybc)

        acc_hi = fast.tile([HI, N], F32, tag="acc_hi")
        acc_lo = fast.tile([LO, N], F32, tag="acc_lo")
        v = st["v"]
        nc.vector.tensor_scalar(out=acc_hi, in0=v[0][0], scalar1=yb[:, 0:1],
                                scalar2=-1.0, op0=ALU.mult, op1=ALU.mult)
        nc.vector.tensor_scalar(out=acc_lo, in0=v[0][1], scalar1=yb[:LO, 0:1],
                                scalar2=-1.0, op0=ALU.mult, op1=ALU.mult)
        for i in range(1, RANK):
            ax_hi = temps.tile([HI, N], F32, tag="ax_hi")
            ax_lo = temps.tile([LO, N], F32, tag="ax_lo")
            nc.vector.tensor_scalar(out=ax_hi, in0=v[i][0], scalar1=yb[:, i : i + 1],
                                    scalar2=None, op0=ALU.mult)
            nc.vector.tensor_scalar(out=ax_lo, in0=v[i][1], scalar1=yb[:LO, i : i + 1],
                                    scalar2=None, op0=ALU.mult)
            nc.gpsimd.tensor_sub(acc_hi, acc_hi, ax_hi)
            nc.gpsimd.tensor_sub(acc_lo, acc_lo, ax_lo)
        nc.sync.dma_start(out=OUT[m, 0:HI, :], in_=acc_hi)
        nc.sync.dma_start(out=OUT[m, HI:N, :], in_=acc_lo)


_NC_CACHE = None


def _get_nc():
    global _NC_CACHE
    if _NC_CACHE is None:
        _NC_CACHE = build_core_kernel()
    return _NC_CACHE


def kernel(D, P, R, max_rank=4, _trace=False):
    D = np.ascontiguousarray(D, dtype=np.float32)
    P = np.ascontiguousarray(P, dtype=np.float32)
    R = np.ascontiguousarray(R, dtype=np.float32)
    nc = _get_nc()
    in_maps = []
    for i in range(NCORES):
        sl = slice(i * MPC, (i + 1) * MPC)
        in_maps.append({"D": D[sl], "P": P[sl], "Rm": R[sl]})
    res = run_bass_kernel_spmd(nc, in_maps, core_ids=list(range(NCORES)),
                               trace=_trace)
    out = np.concatenate([r["OUT"] for r in res.results], axis=0)
    if _trace:
        kernel.last_exec_time_ns = res.exec_time_ns
        kernel.last_trace = res.instructions_and_trace
    return out


if __name__ == "__main__":
    # quick solver self-check
    rng = np.random.default_rng(0)
    A = rng.standard_normal((5, 4, 4)).astype(np.float32)
    M = np.einsum("bij,bkj->bik", A, A) + 4 * np.eye(4, dtype=np.float32)
    cv = rng.standard_normal((5, 4)).astype(np.float32)
    G = np.zeros((5, 14), dtype=np.float32)
    order = [(0, 0), (1, 0), (1, 1), (2, 0), (2, 1), (2, 2), (3, 0), (3, 1), (3, 2), (3, 3)]
    for ix, (k, j) in enumerate(order):
        G[:, ix] = M[:, k, j]
    G[:, 10:] = cv
    y = _solve_sym4_np(G)
    yref = np.stack([np.linalg.solve(M[i], cv[i]) for i in range(5)])
    print("solver max err:", np.abs(y - yref).max())



# revision 8
# speedup vs baseline: 2.9991x; 2.9991x over previous
"""XL-BOMD rank-4 Krylov propagation (EnergyXL) on 8 TRN2 NeuronCores.

Moment-based reformulation: the reference's Gram-Schmidt + rank-4 solve
collapses (exactly, in real arithmetic) to

    out = sum_k delta_k C_k,   C_k = R^k dDS R^k  (pure power sandwiches)

where delta = -L4 @ gamma, G' gamma = c', G'[i][j] = m_{i+j+2},
c'[j] = m_{j+1}, and the B-basis moments m_t come from the C-basis
moments mu_t = <C_i, C_j> (i+j = t, Frobenius) by a forward-difference
(binomial) transform; L4 is the C->B basis binomial matrix.  The
operator v -> R v R is self-adjoint, so mu_t depends only on i+j: nine
inner products total.

Per molecule (N=192): 8 bf16 192^3 matmuls (the only O(N^3) work), 8
PSUM->SBUF copies, 9 Frobenius inner products, and a 4-vector
recombination.  Matrices are stored as [96, 384] tiles (rows 0:96 in
free 0:192, rows 96:192 in free 192:384).  Data-parallel: 64 mols/core,
blocks of 16 share a batched 4x4 symmetric solve on mol-partitions.
"""

import sys

sys.path.insert(0, "/opt/trn_rl_repo")

import numpy as np
import ml_dtypes

import concourse.bass as bass
import concourse.bacc as bacc
import concourse.tile as tile
from concourse import mybir
from concourse.bass_utils import run_bass_kernel_spmd

F32 = mybir.dt.float32
BF16 = mybir.dt.bfloat16
ALU = mybir.AluOpType
ACTF = mybir.ActivationFunctionType

NMOL, N, RANK = 512, 192, 4
NCORES = 8
MPC = NMOL // NCORES      # 64 molecules per core
H, F = 96, 384            # [96, 384] tile layout for a 192x192 matrix
BLK = 16                  # molecules per solve block

# mu_t = <C_i, C_j> pairing per moment column t (i + j = t)
MUPAIR = [(0, 0), (0, 1), (1, 1), (1, 2), (2, 2), (2, 3), (3, 3), (3, 4),
          (4, 4)]
# reduce-engine per moment: diag on ACT = fused Square+accum (1 op);
# off-diag muls always DVE, reduce spread across engines for balance
MOM_RED = {0: "act", 1: "act_acc", 2: "act", 3: "dve", 4: "act", 5: "dve",
           6: "act", 7: "dve", 8: "act"}
# psum->sbuf copy engines: 4 T-copies then 4 C-copies
ENG_TCOPY = ["act", "act", "act", "dve"]
ENG_CCOPY = ["act", "dve", "dve", "dve"]
# forward-difference table offsets: V_t occupies cols OFS[t] .. OFS[t]+(9-t)
OFS = [0, 9, 17, 24, 30, 35, 39, 42, 44]


def _mcol(t):
    """Column of m_t (= first entry of V_t) in the W difference tile."""
    return OFS[t]


def build_core_kernel(n_mols=MPC):
    nc = bacc.Bacc(None, target_bir_lowering=False, enable_partition_id=False)
    D = nc.dram_tensor("D", [n_mols, N, N], BF16, kind="ExternalInput")
    P = nc.dram_tensor("P", [n_mols, N, N], BF16, kind="ExternalInput")
    R = nc.dram_tensor("Rm", [n_mols, N, N], BF16, kind="ExternalInput")
    OUT = nc.dram_tensor("OUT", [n_mols, N, N], F32, kind="ExternalOutput")
    with tile.TileContext(nc) as tc:
        _body(nc, tc, D, P, R, OUT)
    nc.finalize()
    return nc


def _mm_sandwich(nc, ps, lhsT, rhs):
    """ps[96,384] (psum) = lhsT^T @ rhs for 192x192 operands in [96,384]
    layout. lhsT must be symmetric-as-stored (we always pass symmetric
    matrices as lhsT)."""
    nc.tensor.matmul(ps[:, 0:192], lhsT=lhsT[:, 0:96], rhs=rhs[:, 0:192],
                     start=True, stop=False)
    nc.tensor.matmul(ps[:, 0:192], lhsT=lhsT[:, 192:288], rhs=rhs[:, 192:384],
                     start=False, stop=True)
    nc.tensor.matmul(ps[:, 192:384], lhsT=lhsT[:, 96:192], rhs=rhs[:, 0:192],
                     start=True, stop=False)
    nc.tensor.matmul(ps[:, 192:384], lhsT=lhsT[:, 288:384],
                     rhs=rhs[:, 192:384], start=False, stop=True)


def _body(nc, tc, D, P, R, OUT):
    import contextlib

    ctx = contextlib.ExitStack()
    with ctx:
        consts = ctx.enter_context(tc.tile_pool(name="consts", bufs=1))
        inp = ctx.enter_context(tc.tile_pool(name="inp", bufs=4))
        cper = ctx.enter_context(tc.tile_pool(name="cper", bufs=BLK + 5))
        cshort = ctx.enter_context(tc.tile_pool(name="cshort", bufs=4))
        junk = ctx.enter_context(tc.tile_pool(name="junk", bufs=4))
        parts = ctx.enter_context(tc.tile_pool(name="parts", bufs=BLK + 5))
        comb = ctx.enter_context(tc.tile_pool(name="comb", bufs=4))
        blkp = ctx.enter_context(tc.tile_pool(name="blkp", bufs=2))
        ps_mm = ctx.enter_context(tc.tile_pool(name="ps_mm", bufs=2,
                                               space="PSUM"))
        ps_g = ctx.enter_context(tc.tile_pool(name="ps_g", bufs=2,
                                              space="PSUM"))
        ps_bc = ctx.enter_context(tc.tile_pool(name="ps_bc", bufs=2,
                                               space="PSUM"))

        # --- constants ---
        sel = consts.tile([H, 2 * BLK - 1], F32)   # windowed one-hot column
        nc.vector.memset(sel, 0.0)
        nc.vector.memset(sel[:, BLK - 1 : BLK], 1.0)
        ones = consts.tile([H, H], F32)            # bcast lhsT (rows 0:16)
        nc.vector.memset(ones, 1.0)
        id16 = consts.tile([BLK, BLK], F32)
        idt = consts.tile([BLK, BLK], mybir.dt.int32)
        nc.gpsimd.iota(idt, pattern=[[-1, BLK]], base=0, channel_multiplier=1)
        nc.vector.tensor_scalar(out=id16, in0=idt, scalar1=0, scalar2=None,
                                op0=ALU.is_equal)

        n_mols = D.shape[0]
        for b in range(n_mols // BLK):
            mols = list(range(b * BLK, (b + 1) * BLK))
            st = [_mol_chain(nc, D, P, R, m, inp, cper, cshort, junk, parts,
                             ps_mm) for m in mols]
            dbc = _block_tail(nc, b, st, consts, blkp, ps_g, ps_bc, sel, ones,
                              id16)
            for j, (m, s) in enumerate(zip(mols, st)):
                _combo(nc, OUT, m, j, s, dbc, comb)


def _mol_chain(nc, D, P, R, m, inp, cper, cshort, junk, parts, ps_mm):
    """Emit one molecule's power chain + moment accumulations."""
    d_t = inp.tile([H, F], BF16, tag="d_in")
    p_t = inp.tile([H, F], BF16, tag="p_in")
    r_t = inp.tile([H, F], BF16, tag="r_in")
    for tile_, src in ((d_t, D), (p_t, P), (r_t, R)):
        nc.sync.dma_start(out=tile_[:, 0:192], in_=src[m, 0:H, :])
        nc.sync.dma_start(out=tile_[:, 192:384], in_=src[m, H:N, :])

    part = parts.tile([H, 12], F32, tag="part")

    # C0 = D - P (bf16, DVE 2x)
    c = [None] * (RANK + 1)
    c[0] = cper.tile([H, F], BF16, tag="c0", name="c0")
    nc.vector.tensor_sub(c[0], d_t, p_t)

    for i in range(1, RANK + 1):
        # T = C_{i-1} R
        psT = ps_mm.tile([H, F], F32, tag="pT")
        _mm_sandwich(nc, psT, c[i - 1], r_t)
        t_t = cshort.tile([H, F], BF16, tag="t")
        if ENG_TCOPY[i - 1] == "act":
            nc.scalar.copy(t_t, psT)
        else:
            nc.vector.tensor_copy(t_t, psT)
        # C_i = R T
        psC = ps_mm.tile([H, F], F32, tag="pC")
        _mm_sandwich(nc, psC, r_t, t_t)
        pool = cper if i < RANK else cshort
        c[i] = pool.tile([H, F], BF16, tag=f"c{i}", name=f"c{i}")
        if ENG_CCOPY[i - 1] == "act":
            nc.scalar.copy(c[i], psC)
        else:
            nc.vector.tensor_copy(c[i], psC)

    # moments mu_t -> part[:, t]
    # diag (i==j): one ACT Square+accum.  off-diag: DVE mul (bf16 2x) into
    # junk scratch, then a reduce on the engine given by MOM_RED.
    for t, (i, j) in enumerate(MUPAIR):
        if i == j and MOM_RED[t] == "act":
            ja = junk.tile([H, F], BF16, tag="junk_a")
            nc.scalar.activation(out=ja, in_=c[i], func=ACTF.Square,
                                 accum_out=part[:, t : t + 1])
            continue
        jd = junk.tile([H, F], BF16, tag="junk_d")
        nc.vector.tensor_mul(jd, c[i], c[j])
        red = MOM_RED[t]
        if red == "act_acc":
            ja2 = junk.tile([H, F], BF16, tag="junk_a2")
            nc.scalar.activation(out=ja2, in_=jd, func=ACTF.Copy,
                                 accum_out=part[:, t : t + 1])
        else:
            nc.vector.tensor_reduce(out=part[:, t : t + 1], in_=jd,
                                    axis=mybir.AxisListType.X, op=ALU.add)

    return {"part": part, "c": c[: RANK]}


def _block_tail(nc, b, st, consts, blkp, ps_g, ps_bc, sel, ones, id16):
    """Gather moments, difference-transform, solve, broadcast deltas."""
    gps = ps_g.tile([BLK, 9], F32, tag="g")
    for j, s in enumerate(st):
        nc.tensor.matmul(gps, lhsT=sel[:, BLK - 1 - j : 2 * BLK - 1 - j],
                         rhs=s["part"][:, 0:9], start=(j == 0),
                         stop=(j == len(st) - 1))

    W = blkp.tile([BLK, 45], F32, tag="W")
    nc.vector.tensor_copy(W[:, 0:9], gps)
    # forward differences: V_t[k] = V_{t-1}[k] - V_{t-1}[k-1], k = t..8
    for t in range(1, 9):
        w = 9 - t
        o, po = OFS[t], OFS[t - 1]
        nc.vector.tensor_sub(W[:, o : o + w], W[:, po + 1 : po + 1 + w],
                             W[:, po : po + w])

    # private copies of solver-overwritten inputs
    S = blkp.tile([BLK, 10], F32, tag="S")
    priv = [4, 5, 6, 6, 7, 8, 1, 2, 3, 4]  # e f h gg i_ jj r0... see below
    # S cols: 0:e<-m4 1:f<-m5 2:h<-m6 3:gg<-m6 4:i_<-m7 5:jj<-m8
    #         6:r0<-m1 7:r1<-m2 8:r2<-m3 9:r3<-m4
    for scol, mt in zip(range(10), (4, 5, 6, 6, 7, 8, 1, 2, 3, 4)):
        nc.vector.tensor_copy(S[:, scol : scol + 1],
                              W[:, _mcol(mt) : _mcol(mt) + 1])

    X = blkp.tile([BLK, 16], F32, tag="X")
    ys = _solve_sym4(nc, W, S, X)

    # delta = -L4 @ gamma  (C-basis output coefficients)
    DL = blkp.tile([BLK, 4], F32, tag="DL")
    y0, y1, y2, y3 = ys
    ta = X[:, 12:13]
    u = X[:, 13:14]
    v = X[:, 14:15]
    w2 = X[:, 15:16]
    mul = nc.vector.tensor_mul
    sub = nc.vector.tensor_sub
    add = nc.vector.tensor_add
    ts = nc.vector.tensor_scalar
    ts(out=DL[:, 3:4], in0=y3, scalar1=-1.0, scalar2=None, op0=ALU.mult)
    ts(out=ta, in0=y3, scalar1=3.0, scalar2=None, op0=ALU.mult)
    sub(DL[:, 2:3], ta, y2)                      # 3*y3 - y2
    ts(out=u, in0=y2, scalar1=2.0, scalar2=None, op0=ALU.mult)
    sub(v, u, y1)                                # 2*y2 - y1
    sub(DL[:, 1:2], v, ta)                       # 2*y2 - y1 - 3*y3
    sub(w2, y1, y0)
    sub(u, y3, y2)
    add(DL[:, 0:1], w2, u)                       # y1 - y0 + y3 - y2

    # broadcast deltas: dbc[:, 4j+k] = delta_k of mol j, on 96 partitions
    bcps = ps_bc.tile([H, 4 * BLK], F32, tag="bc")
    for j in range(BLK):
        mk = blkp.tile([BLK, 4], F32, tag="mk")
        ts(out=mk, in0=DL, scalar1=id16[:, j : j + 1], scalar2=None,
           op0=ALU.mult)
        nc.tensor.matmul(bcps[:, 4 * j : 4 * j + 4], lhsT=ones[0:BLK, 0:H],
                         rhs=mk, start=True, stop=True)
    dbc = blkp.tile([H, 4 * BLK], F32, tag="dbc")
    nc.scalar.copy(dbc, bcps)
    return dbc


def _combo(nc, OUT, m, j, s, dbc, comb):
    """out = sum_k delta_k C_k, then DMA to HBM."""
    c = s["c"]
    ts = nc.vector.tensor_scalar
    tsp = nc.gpsimd.tensor_scalar
    tk = [comb.tile([H, F], BF16, tag=f"tk{k}", name=f"tk{k}")
          for k in range(4)]
    for k in range(4):
        eng = ts if k < 2 else tsp
        eng(out=tk[k], in0=c[k], scalar1=dbc[:, 4 * j + k : 4 * j + k + 1],
            scalar2=None, op0=ALU.mult)
    u01 = comb.tile([H, F], BF16, tag="u01")
    nc.vector.tensor_add(u01, tk[0], tk[1])
    u23 = comb.tile([H, F], BF16, tag="u23")
    nc.gpsimd.tensor_add(u23, tk[2], tk[3])
    os_ = comb.tile([H, F], F32, tag="os")
    nc.gpsimd.tensor_add(os_, u01, u23)
    nc.sync.dma_start(out=OUT[m, 0:H, :], in_=os_[:, 0:192])
    nc.sync.dma_start(out=OUT[m, H:N, :], in_=os_[:, 192:384])


def _solve_sym4(nc, W, S, X):
    """Batched 4x4 symmetric solve on [BLK,1] column APs.

    Hankel inputs: O_ij = m_{i+j+2} (views into W), rhs c_j = m_{j+1}.
    Overwritten entries live in S (private copies); X is scratch.
    Returns [y0..y3] column APs (in X cols 8..11).
    Mirrors _solve_sym4_np below; keep in sync."""
    def wm(t):
        return W[:, _mcol(t) : _mcol(t) + 1]

    def sc(i):
        return S[:, i : i + 1]

    a, bb, cc, dd = wm(2), wm(3), wm(4), wm(5)       # read-only
    e, f, h, gg, i_, jj = (sc(k) for k in range(6))  # overwritten
    r0, r1, r2, r3 = (sc(k) for k in range(6, 10))
    p0, p1, p2, p3 = (X[:, k : k + 1] for k in range(4))
    l1, l2, l3 = (X[:, k : k + 1] for k in range(4, 7))
    t0 = X[:, 7:8]
    y0, y1, y2, y3 = (X[:, k : k + 1] for k in range(8, 12))

    mul = nc.vector.tensor_mul
    sub = nc.vector.tensor_sub
    rec = nc.vector.reciprocal

    def upd(x, l, src):  # x -= l*src
        mul(t0, l, src)
        sub(x, x, t0)

    rec(p0, a)
    mul(l1, bb, p0); mul(l2, cc, p0); mul(l3, dd, p0)
    upd(e, l1, bb); upd(f, l2, bb); upd(gg, l3, bb)
    upd(h, l2, cc); upd(i_, l3, cc); upd(jj, l3, dd)
    upd(r1, l1, r0); upd(r2, l2, r0); upd(r3, l3, r0)

    rec(p1, e)
    mul(l2, f, p1); mul(l3, gg, p1)
    upd(h, l2, f); upd(i_, l3, f); upd(jj, l3, gg)
    upd(r2, l2, r1); upd(r3, l3, r1)

    rec(p2, h)
    mul(l3, i_, p2)
    upd(jj, l3, i_); upd(r3, l3, r2)

    rec(p3, jj)
    mul(y3, r3, p3)
    upd(r2, i_, y3); mul(y2, r2, p2)
    upd(r1, f, y2); upd(r1, gg, y3); mul(y1, r1, p1)
    upd(r0, bb, y1); upd(r0, cc, y2); upd(r0, dd, y3); mul(y0, r0, p0)
    return [y0, y1, y2, y3]


# ---------------------------------------------------------------------------
# numpy mirror (for verification without hardware)

def _bf(x):
    return np.asarray(x).astype(ml_dtypes.bfloat16).astype(np.float32)


def _solve_sym4_np(m):
    """m: [n, 9] float32 (m[:, t] = m_t, col 0 unused). Returns y [n, 4]."""
    col = lambda t: m[:, t : t + 1].astype(np.float32)
    a, bb, cc, dd = col(2), col(3), col(4), col(5)
    e, f, h, gg, i_, jj = col(4), col(5), col(6), col(6), col(7), col(8)
    r0, r1, r2, r3 = col(1), col(2), col(3), col(4)
    p0 = np.float32(1.0) / a
    l1, l2, l3 = bb * p0, cc * p0, dd * p0
    e = e - l1 * bb; f = f - l2 * bb; gg = gg - l3 * bb
    h = h - l2 * cc; i_ = i_ - l3 * cc; jj = jj - l3 * dd
    r1 = r1 - l1 * r0; r2 = r2 - l2 * r0; r3 = r3 - l3 * r0
    p1 = np.float32(1.0) / e
    l2, l3 = f * p1, gg * p1
    h = h - l2 * f; i_ = i_ - l3 * f; jj = jj - l3 * gg
    r2 = r2 - l2 * r1; r3 = r3 - l3 * r1
    p2 = np.float32(1.0) / h
    l3 = i_ * p2
    jj = jj - l3 * i_; r3 = r3 - l3 * r2
    p3 = np.float32(1.0) / jj
    y3 = r3 * p3
    r2 = r2 - i_ * y3; y2 = r2 * p2
    r1 = r1 - f * y2; r1 = r1 - gg * y3; y1 = r1 * p1
    r0 = r0 - bb * y1; r0 = r0 - cc * y2; r0 = r0 - dd * y3; y0 = r0 * p0
    return np.concatenate([y0, y1, y2, y3], axis=1)


def _mirror_numpy(D, P, R):
    """Bit-approximate mirror of the device algorithm (bf16 rounding at the
    same points), for offline validation."""
    Db, Pb, Rb = _bf(D), _bf(P), _bf(R)
    b = D.shape[0]
    C = [None] * (RANK + 1)
    C[0] = _bf(Db - Pb)
    for i in range(1, RANK + 1):
        T = _bf(np.einsum("bij,bjk->bik", C[i - 1], Rb, dtype=np.float32))
        C[i] = _bf(np.einsum("bij,bjk->bik", Rb, T, dtype=np.float32))
    mu = np.zeros((b, 9), dtype=np.float32)
    for t, (i, j) in enumerate(MUPAIR):
        mu[:, t] = np.sum(C[i].astype(np.float32) * C[j].astype(np.float32),
                          axis=(1, 2))
    # forward differences
    V = mu.copy()
    m = np.zeros((b, 9), dtype=np.float32)
    for t in range(1, 9):
        V = (V[:, 1:] - V[:, :-1]).astype(np.float32)
        m[:, t] = V[:, 0]
    y = _solve_sym4_np(m)
    y0, y1, y2, y3 = (y[:, k : k + 1] for k in range(4))
    d3 = -y3
    d2 = 3 * y3 - y2
    d1 = 2 * y2 - y1 - 3 * y3
    d0 = y1 - y0 + y3 - y2
    dl = np.concatenate([d0, d1, d2, d3], axis=1).astype(np.float32)
    t0 = _bf(C[0] * dl[:, 0, None, None])
    t1 = _bf(C[1] * dl[:, 1, None, None])
    t2 = _bf(C[2] * dl[:, 2, None, None])
    t3 = _bf(C[3] * dl[:, 3, None, None])
    u01 = _bf(t0 + t1)
    u23 = _bf(t2 + t3)
    return (u01 + u23).astype(np.float32)


# ---------------------------------------------------------------------------

_NC_CACHE = None


def _get_nc():
    global _NC_CACHE
    if _NC_CACHE is None:
        _NC_CACHE = build_core_kernel()
    return _NC_CACHE


def kernel(D, P, R, max_rank=4, _trace=False):
    BF = ml_dtypes.bfloat16
    D = np.ascontiguousarray(np.asarray(D, dtype=np.float32).astype(BF))
    P = np.ascontiguousarray(np.asarray(P, dtype=np.float32).astype(BF))
    R = np.ascontiguousarray(np.asarray(R, dtype=np.float32).astype(BF))
    nc = _get_nc()
    in_maps = []
    for i in range(NCORES):
        sl = slice(i * MPC, (i + 1) * MPC)
        in_maps.append({"D": D[sl], "P": P[sl], "Rm": R[sl]})
    res = run_bass_kernel_spmd(nc, in_maps, core_ids=list(range(NCORES)),
                               trace=_trace)
    out = np.concatenate([r["OUT"] for r in res.results], axis=0)
    if _trace:
        kernel.last_exec_time_ns = res.exec_time_ns
        kernel.last_trace = res.instructions_and_trace
    return out


if __name__ == "__main__":
    # offline mirror check against the jax reference
    sys.path.insert(0, "/root/problem")
    import jax

    jax.config.update("jax_platforms", "cpu")
    import reference

    inputs = {k: np.asarray(v) for k, v in reference.setup_inputs().items()}
    expected = np.asarray(reference.reference(**reference.setup_inputs()))
    got = _mirror_numpy(inputs["D"], inputs["P"], inputs["R"])
    scale = np.abs(expected).max()
    rel = np.abs(got - expected).max() / scale
    print(f"mirror rel err: {rel:.3e} (scale {scale:.3f})")


# revision 13
# speedup vs baseline: 5.9278x; 1.9765x over previous
"""XL-BOMD rank-4 Krylov propagation (EnergyXL) on 8 TRN2 NeuronCores.

Moment-based reformulation: the reference's Gram-Schmidt + rank-4 solve
collapses (exactly, in real arithmetic) to

    out = sum_k delta_k C_k,   C_k = R^k dDS R^k  (pure power sandwiches)

where delta = -L4 @ gamma, G' gamma = c', G'[i][j] = m_{i+j+2},
c'[j] = m_{j+1}, and the B-basis moments m_t come from the C-basis
moments mu_t = <C_i, C_j> (i+j = t, Frobenius) by a forward-difference
(binomial) transform; L4 is the C->B basis binomial matrix.  The
operator v -> R v R is self-adjoint, so mu_t depends only on i+j: nine
inner products total.

Per molecule (N=192): 8 bf16 192^3 matmuls (the only O(N^3) work), 8
PSUM->SBUF copies, 9 Frobenius inner products, and a 4-vector
recombination.  Matrices are stored as [96, 384] tiles (rows 0:96 in
free 0:192, rows 96:192 in free 192:384).  Data-parallel: 64 mols/core,
blocks of 16 share a batched 4x4 symmetric solve on mol-partitions.
"""

import sys

sys.path.insert(0, "/opt/trn_rl_repo")

import numpy as np
import ml_dtypes

import concourse.bass as bass
import concourse.bacc as bacc
import concourse.tile as tile
from concourse import mybir
from concourse.bass_utils import run_bass_kernel_spmd

F32 = mybir.dt.float32
BF16 = mybir.dt.bfloat16
ALU = mybir.AluOpType
ACTF = mybir.ActivationFunctionType

NMOL, N, RANK = 512, 192, 4
NCORES = 8
MPC = NMOL // NCORES      # 64 molecules per core
H, F = 96, 384            # [96, 384] tile layout for a 192x192 matrix
BLK = 16                  # molecules per solve block

# mu_t = <C_i, C_j> pairing per moment column t (i + j = t)
MUPAIR = [(0, 0), (0, 1), (1, 1), (1, 2), (2, 2), (2, 3), (3, 3), (3, 4),
          (4, 4)]
# reduce-engine per moment: diag on ACT = fused Square+accum (1 op);
# off-diag muls always DVE, reduce spread across engines for balance
MOM_RED = {0: "act", 1: "act_acc", 2: "act", 3: "dve", 4: "act", 5: "dve",
           6: "act", 7: "dve", 8: "act"}
# psum->sbuf copy engines: 4 T-copies then 4 C-copies
ENG_TCOPY = ["act", "act", "act", "dve"]
ENG_CCOPY = ["act", "dve", "dve", "dve"]
# forward-difference table offsets: V_t occupies cols OFS[t] .. OFS[t]+(9-t)
OFS = [0, 9, 17, 24, 30, 35, 39, 42, 44]


def _mcol(t):
    """Column of m_t (= first entry of V_t) in the W difference tile."""
    return OFS[t]


def build_core_kernel(n_mols=MPC):
    nc = bacc.Bacc(None, target_bir_lowering=False, enable_partition_id=False)
    D = nc.dram_tensor("D", [n_mols, N, N], BF16, kind="ExternalInput")
    P = nc.dram_tensor("P", [n_mols, N, N], BF16, kind="ExternalInput")
    R = nc.dram_tensor("Rm", [n_mols, N, N], BF16, kind="ExternalInput")
    OUT = nc.dram_tensor("OUT", [n_mols, N, N], F32, kind="ExternalOutput")
    with tile.TileContext(nc) as tc:
        _body(nc, tc, D, P, R, OUT)
    nc.finalize()
    return nc


def _mm_sandwich(nc, ps, lhsT, rhs):
    """ps[96,384] (psum) = lhsT^T @ rhs for 192x192 operands in [96,384]
    layout. lhsT must be symmetric-as-stored (we always pass symmetric
    matrices as lhsT)."""
    nc.tensor.matmul(ps[:, 0:192], lhsT=lhsT[:, 0:96], rhs=rhs[:, 0:192],
                     start=True, stop=False)
    nc.tensor.matmul(ps[:, 0:192], lhsT=lhsT[:, 192:288], rhs=rhs[:, 192:384],
                     start=False, stop=True)
    nc.tensor.matmul(ps[:, 192:384], lhsT=lhsT[:, 96:192], rhs=rhs[:, 0:192],
                     start=True, stop=False)
    nc.tensor.matmul(ps[:, 192:384], lhsT=lhsT[:, 288:384],
                     rhs=rhs[:, 192:384], start=False, stop=True)


def _body(nc, tc, D, P, R, OUT):
    import contextlib

    ctx = contextlib.ExitStack()
    with ctx:
        consts = ctx.enter_context(tc.tile_pool(name="consts", bufs=1))
        inp = ctx.enter_context(tc.tile_pool(name="inp", bufs=4))
        cper = ctx.enter_context(tc.tile_pool(name="cper", bufs=BLK + 5))
        cshort = ctx.enter_context(tc.tile_pool(name="cshort", bufs=4))
        junk = ctx.enter_context(tc.tile_pool(name="junk", bufs=4))
        parts = ctx.enter_context(tc.tile_pool(name="parts", bufs=BLK + 5))
        comb = ctx.enter_context(tc.tile_pool(name="comb", bufs=4))
        blkp = ctx.enter_context(tc.tile_pool(name="blkp", bufs=2))
        ps_mm = ctx.enter_context(tc.tile_pool(name="ps_mm", bufs=2,
                                               space="PSUM"))
        ps_g = ctx.enter_context(tc.tile_pool(name="ps_g", bufs=1,
                                              space="PSUM"))
        ps_bc = ctx.enter_context(tc.tile_pool(name="ps_bc", bufs=1,
                                               space="PSUM"))
        ps_o = ctx.enter_context(tc.tile_pool(name="ps_o", bufs=2,
                                              space="PSUM"))

        # --- constants ---
        sel = consts.tile([H, 2 * BLK - 1], F32)   # windowed one-hot column
        nc.vector.memset(sel, 0.0)
        nc.vector.memset(sel[:, BLK - 1 : BLK], 1.0)
        ones = consts.tile([H, H], F32)            # bcast lhsT (rows 0:16)
        nc.vector.memset(ones, 1.0)
        id16 = consts.tile([BLK, BLK], F32)
        idt = consts.tile([BLK, BLK], mybir.dt.int32)
        nc.gpsimd.iota(idt, pattern=[[-1, BLK]], base=0, channel_multiplier=1)
        nc.vector.tensor_scalar(out=id16, in0=idt, scalar1=0, scalar2=None,
                                op0=ALU.is_equal)
        i96 = consts.tile([H, H], BF16)            # identity, combo lhsT seed
        idt96 = consts.tile([H, H], mybir.dt.int32)
        nc.gpsimd.iota(idt96, pattern=[[-1, H]], base=0, channel_multiplier=1)
        nc.vector.tensor_scalar(out=i96, in0=idt96, scalar1=0, scalar2=None,
                                op0=ALU.is_equal)

        n_mols = D.shape[0]
        for b in range(n_mols // BLK):
            mols = list(range(b * BLK, (b + 1) * BLK))
            st = [_mol_chain(nc, D, P, R, m, inp, cper, cshort, junk, parts,
                             ps_mm) for m in mols]
            dbc = _block_tail(nc, b, st, consts, blkp, ps_g, ps_bc, sel, ones,
                              id16)
            for j, (m, s) in enumerate(zip(mols, st)):
                _combo(nc, OUT, m, j, s, dbc, comb, ps_o, i96)


def _mol_chain(nc, D, P, R, m, inp, cper, cshort, junk, parts, ps_mm):
    """Emit one molecule's power chain + moment accumulations."""
    d_t = inp.tile([H, F], BF16, tag="d_in")
    p_t = inp.tile([H, F], BF16, tag="p_in")
    r_t = inp.tile([H, F], BF16, tag="r_in")
    for tile_, src in ((d_t, D), (p_t, P), (r_t, R)):
        nc.sync.dma_start(out=tile_[:, 0:192], in_=src[m, 0:H, :])
        nc.sync.dma_start(out=tile_[:, 192:384], in_=src[m, H:N, :])

    part = parts.tile([H, 12], F32, tag="part")

    # C0 = D - P (bf16, DVE 2x)
    c = [None] * (RANK + 1)
    c[0] = cper.tile([H, F], BF16, tag="c0", name="c0")
    nc.vector.tensor_sub(c[0], d_t, p_t)

    for i in range(1, RANK + 1):
        # T = C_{i-1} R
        psT = ps_mm.tile([H, F], F32, tag="pT")
        _mm_sandwich(nc, psT, c[i - 1], r_t)
        t_t = cshort.tile([H, F], BF16, tag="t")
        if ENG_TCOPY[i - 1] == "act":
            nc.scalar.copy(t_t, psT)
        else:
            nc.vector.tensor_copy(t_t, psT)
        # C_i = R T
        psC = ps_mm.tile([H, F], F32, tag="pC")
        _mm_sandwich(nc, psC, r_t, t_t)
        pool = cper if i < RANK else cshort
        c[i] = pool.tile([H, F], BF16, tag=f"c{i}", name=f"c{i}")
        if ENG_CCOPY[i - 1] == "act":
            nc.scalar.copy(c[i], psC)
        else:
            nc.vector.tensor_copy(c[i], psC)

    # moments mu_t -> part[:, t]
    # diag (i==j): one ACT Square+accum.  off-diag: DVE mul (bf16 2x) into
    # junk scratch, then a reduce on the engine given by MOM_RED.
    for t, (i, j) in enumerate(MUPAIR):
        if i == j and MOM_RED[t] == "act":
            ja = junk.tile([H, F], BF16, tag="junk_a")
            nc.scalar.activation(out=ja, in_=c[i], func=ACTF.Square,
                                 accum_out=part[:, t : t + 1])
            continue
        jd = junk.tile([H, F], BF16, tag="junk_d")
        nc.vector.tensor_mul(jd, c[i], c[j])
        red = MOM_RED[t]
        if red == "act_acc":
            ja2 = junk.tile([H, F], BF16, tag="junk_a2")
            nc.scalar.activation(out=ja2, in_=jd, func=ACTF.Copy,
                                 accum_out=part[:, t : t + 1])
        else:
            nc.vector.tensor_reduce(out=part[:, t : t + 1], in_=jd,
                                    axis=mybir.AxisListType.X, op=ALU.add)

    return {"part": part, "c": c[: RANK]}


def _block_tail(nc, b, st, consts, blkp, ps_g, ps_bc, sel, ones, id16):
    """Gather moments, difference-transform, solve, broadcast deltas."""
    gps = ps_g.tile([BLK, 9], F32, tag="g")
    for j, s in enumerate(st):
        nc.tensor.matmul(gps, lhsT=sel[:, BLK - 1 - j : 2 * BLK - 1 - j],
                         rhs=s["part"][:, 0:9], start=(j == 0),
                         stop=(j == len(st) - 1))

    W = blkp.tile([BLK, 45], F32, tag="W")
    nc.vector.tensor_copy(W[:, 0:9], gps)
    # forward differences: V_t[k] = V_{t-1}[k] - V_{t-1}[k-1], k = t..8
    for t in range(1, 9):
        w = 9 - t
        o, po = OFS[t], OFS[t - 1]
        nc.vector.tensor_sub(W[:, o : o + w], W[:, po + 1 : po + 1 + w],
                             W[:, po : po + w])

    # private copies of solver-overwritten inputs
    S = blkp.tile([BLK, 10], F32, tag="S")
    priv = [4, 5, 6, 6, 7, 8, 1, 2, 3, 4]  # e f h gg i_ jj r0... see below
    # S cols: 0:e<-m4 1:f<-m5 2:h<-m6 3:gg<-m6 4:i_<-m7 5:jj<-m8
    #         6:r0<-m1 7:r1<-m2 8:r2<-m3 9:r3<-m4
    for scol, mt in zip(range(10), (4, 5, 6, 6, 7, 8, 1, 2, 3, 4)):
        nc.vector.tensor_copy(S[:, scol : scol + 1],
                              W[:, _mcol(mt) : _mcol(mt) + 1])

    X = blkp.tile([BLK, 16], F32, tag="X")
    ys = _solve_sym4(nc, W, S, X)

    # delta = -L4 @ gamma  (C-basis output coefficients)
    DL = blkp.tile([BLK, 4], F32, tag="DL")
    y0, y1, y2, y3 = ys
    ta = X[:, 12:13]
    u = X[:, 13:14]
    v = X[:, 14:15]
    w2 = X[:, 15:16]
    mul = nc.vector.tensor_mul
    sub = nc.vector.tensor_sub
    add = nc.vector.tensor_add
    ts = nc.vector.tensor_scalar
    ts(out=DL[:, 3:4], in0=y3, scalar1=-1.0, scalar2=None, op0=ALU.mult)
    ts(out=ta, in0=y3, scalar1=3.0, scalar2=None, op0=ALU.mult)
    sub(DL[:, 2:3], ta, y2)                      # 3*y3 - y2
    ts(out=u, in0=y2, scalar1=2.0, scalar2=None, op0=ALU.mult)
    sub(v, u, y1)                                # 2*y2 - y1
    sub(DL[:, 1:2], v, ta)                       # 2*y2 - y1 - 3*y3
    sub(w2, y1, y0)
    sub(u, y3, y2)
    add(DL[:, 0:1], w2, u)                       # y1 - y0 + y3 - y2

    # broadcast deltas: dbc[:, 4j+k] = delta_k of mol j, on 96 partitions
    bcps = ps_bc.tile([H, 4 * BLK], F32, tag="bc")
    for j in range(BLK):
        mk = blkp.tile([BLK, 4], F32, tag="mk")
        ts(out=mk, in0=DL, scalar1=id16[:, j : j + 1], scalar2=None,
           op0=ALU.mult)
        nc.tensor.matmul(bcps[:, 4 * j : 4 * j + 4], lhsT=ones[0:BLK, 0:H],
                         rhs=mk, start=True, stop=True)
    dbc = blkp.tile([H, 4 * BLK], F32, tag="dbc")
    nc.scalar.copy(dbc, bcps)
    return dbc


def _combo(nc, OUT, m, j, s, dbc, comb, ps_o, i96):
    """out = sum_k delta_k C_k via PE accumulation of (delta_k I) @ C_k."""
    c = s["c"]
    ts = nc.vector.tensor_scalar
    dg = [comb.tile([H, H], BF16, tag=f"dg{k}", name=f"dg{k}")
          for k in range(4)]
    for k in range(4):
        ts(out=dg[k], in0=i96, scalar1=dbc[:, 4 * j + k : 4 * j + k + 1],
           scalar2=None, op0=ALU.mult)
    pso = ps_o.tile([H, F], F32, tag="po")
    for k in range(4):
        nc.tensor.matmul(pso, lhsT=dg[k], rhs=c[k], start=(k == 0),
                         stop=(k == 3))
    os_ = comb.tile([H, F], F32, tag="os")
    nc.scalar.copy(os_, pso)
    nc.sync.dma_start(out=OUT[m, 0:H, :], in_=os_[:, 0:192])
    nc.sync.dma_start(out=OUT[m, H:N, :], in_=os_[:, 192:384])


def _solve_sym4(nc, W, S, X):
    """Batched 4x4 symmetric solve on [BLK,1] column APs.

    Hankel inputs: O_ij = m_{i+j+2} (views into W), rhs c_j = m_{j+1}.
    Overwritten entries live in S (private copies); X is scratch.
    Returns [y0..y3] column APs (in X cols 8..11).
    Mirrors _solve_sym4_np below; keep in sync."""
    def wm(t):
        return W[:, _mcol(t) : _mcol(t) + 1]

    def sc(i):
        return S[:, i : i + 1]

    a, bb, cc, dd = wm(2), wm(3), wm(4), wm(5)       # read-only
    e, f, h, gg, i_, jj = (sc(k) for k in range(6))  # overwritten
    r0, r1, r2, r3 = (sc(k) for k in range(6, 10))
    p0, p1, p2, p3 = (X[:, k : k + 1] for k in range(4))
    l1, l2, l3 = (X[:, k : k + 1] for k in range(4, 7))
    t0 = X[:, 7:8]
    y0, y1, y2, y3 = (X[:, k : k + 1] for k in range(8, 12))

    mul = nc.vector.tensor_mul
    sub = nc.vector.tensor_sub
    rec = nc.vector.reciprocal

    def upd(x, l, src):  # x -= l*src
        mul(t0, l, src)
        sub(x, x, t0)

    rec(p0, a)
    mul(l1, bb, p0); mul(l2, cc, p0); mul(l3, dd, p0)
    upd(e, l1, bb); upd(f, l2, bb); upd(gg, l3, bb)
    upd(h, l2, cc); upd(i_, l3, cc); upd(jj, l3, dd)
    upd(r1, l1, r0); upd(r2, l2, r0); upd(r3, l3, r0)

    rec(p1, e)
    mul(l2, f, p1); mul(l3, gg, p1)
    upd(h, l2, f); upd(i_, l3, f); upd(jj, l3, gg)
    upd(r2, l2, r1); upd(r3, l3, r1)

    rec(p2, h)
    mul(l3, i_, p2)
    upd(jj, l3, i_); upd(r3, l3, r2)

    rec(p3, jj)
    mul(y3, r3, p3)
    upd(r2, i_, y3); mul(y2, r2, p2)
    upd(r1, f, y2); upd(r1, gg, y3); mul(y1, r1, p1)
    upd(r0, bb, y1); upd(r0, cc, y2); upd(r0, dd, y3); mul(y0, r0, p0)
    return [y0, y1, y2, y3]


# ---------------------------------------------------------------------------
# numpy mirror (for verification without hardware)

def _bf(x):
    return np.asarray(x).astype(ml_dtypes.bfloat16).astype(np.float32)


def _solve_sym4_np(m):
    """m: [n, 9] float32 (m[:, t] = m_t, col 0 unused). Returns y [n, 4]."""
    col = lambda t: m[:, t : t + 1].astype(np.float32)
    a, bb, cc, dd = col(2), col(3), col(4), col(5)
    e, f, h, gg, i_, jj = col(4), col(5), col(6), col(6), col(7), col(8)
    r0, r1, r2, r3 = col(1), col(2), col(3), col(4)
    p0 = np.float32(1.0) / a
    l1, l2, l3 = bb * p0, cc * p0, dd * p0
    e = e - l1 * bb; f = f - l2 * bb; gg = gg - l3 * bb
    h = h - l2 * cc; i_ = i_ - l3 * cc; jj = jj - l3 * dd
    r1 = r1 - l1 * r0; r2 = r2 - l2 * r0; r3 = r3 - l3 * r0
    p1 = np.float32(1.0) / e
    l2, l3 = f * p1, gg * p1
    h = h - l2 * f; i_ = i_ - l3 * f; jj = jj - l3 * gg
    r2 = r2 - l2 * r1; r3 = r3 - l3 * r1
    p2 = np.float32(1.0) / h
    l3 = i_ * p2
    jj = jj - l3 * i_; r3 = r3 - l3 * r2
    p3 = np.float32(1.0) / jj
    y3 = r3 * p3
    r2 = r2 - i_ * y3; y2 = r2 * p2
    r1 = r1 - f * y2; r1 = r1 - gg * y3; y1 = r1 * p1
    r0 = r0 - bb * y1; r0 = r0 - cc * y2; r0 = r0 - dd * y3; y0 = r0 * p0
    return np.concatenate([y0, y1, y2, y3], axis=1)


def _mirror_numpy(D, P, R):
    """Bit-approximate mirror of the device algorithm (bf16 rounding at the
    same points), for offline validation."""
    Db, Pb, Rb = _bf(D), _bf(P), _bf(R)
    b = D.shape[0]
    C = [None] * (RANK + 1)
    C[0] = _bf(Db - Pb)
    for i in range(1, RANK + 1):
        T = _bf(np.einsum("bij,bjk->bik", C[i - 1], Rb, dtype=np.float32))
        C[i] = _bf(np.einsum("bij,bjk->bik", Rb, T, dtype=np.float32))
    mu = np.zeros((b, 9), dtype=np.float32)
    for t, (i, j) in enumerate(MUPAIR):
        mu[:, t] = np.sum(C[i].astype(np.float32) * C[j].astype(np.float32),
                          axis=(1, 2))
    # forward differences
    V = mu.copy()
    m = np.zeros((b, 9), dtype=np.float32)
    for t in range(1, 9):
        V = (V[:, 1:] - V[:, :-1]).astype(np.float32)
        m[:, t] = V[:, 0]
    y = _solve_sym4_np(m)
    y0, y1, y2, y3 = (y[:, k : k + 1] for k in range(4))
    d3 = -y3
    d2 = 3 * y3 - y2
    d1 = 2 * y2 - y1 - 3 * y3
    d0 = y1 - y0 + y3 - y2
    dl = _bf(np.concatenate([d0, d1, d2, d3], axis=1))
    # PE combo: (delta_k I)_bf16 @ C_k accumulated in fp32 PSUM
    return sum(dl[:, k, None, None] * C[k] for k in range(4)).astype(np.float32)


# ---------------------------------------------------------------------------

_NC_CACHE = None


def _get_nc():
    global _NC_CACHE
    if _NC_CACHE is None:
        _NC_CACHE = build_core_kernel()
    return _NC_CACHE


def kernel(D, P, R, max_rank=4, _trace=False):
    BF = ml_dtypes.bfloat16
    D = np.ascontiguousarray(np.asarray(D, dtype=np.float32).astype(BF))
    P = np.ascontiguousarray(np.asarray(P, dtype=np.float32).astype(BF))
    R = np.ascontiguousarray(np.asarray(R, dtype=np.float32).astype(BF))
    nc = _get_nc()
    in_maps = []
    for i in range(NCORES):
        sl = slice(i * MPC, (i + 1) * MPC)
        in_maps.append({"D": D[sl], "P": P[sl], "Rm": R[sl]})
    res = run_bass_kernel_spmd(nc, in_maps, core_ids=list(range(NCORES)),
                               trace=_trace)
    out = np.concatenate([r["OUT"] for r in res.results], axis=0)
    if _trace:
        kernel.last_exec_time_ns = res.exec_time_ns
        kernel.last_trace = res.instructions_and_trace
    return out


if __name__ == "__main__":
    # offline mirror check against the jax reference
    sys.path.insert(0, "/root/problem")
    import jax

    jax.config.update("jax_platforms", "cpu")
    import reference

    inputs = {k: np.asarray(v) for k, v in reference.setup_inputs().items()}
    expected = np.asarray(reference.reference(**reference.setup_inputs()))
    got = _mirror_numpy(inputs["D"], inputs["P"], inputs["R"])
    scale = np.abs(expected).max()
    rel = np.abs(got - expected).max() / scale
    print(f"mirror rel err: {rel:.3e} (scale {scale:.3f})")
